# revision 6
# baseline (speedup 1.0000x reference)
"""Trainium2 Bass kernel for nn_Attention_77060303225320.

Multi-head attention (B=16, S=1024, D=1024, 16 heads, head_dim 64) with mixed
1D (latent, 16 dims) + 2D (spatial, 48 dims) RoPE, softmax, and output
projection.

Sharding: data-parallel over batch — 2 batches per core on 8 NeuronCores,
weights replicated, zero communication.

Per-core dataflow (all matmuls in float32r — full-rate fp32 on the PE):
  - x is DMA'd naturally, transposed on the PE (128x128 tiles) into xT [d, s].
  - qT/kT computed as W^T-stationary matmuls into PSUM [j, s]; biases are
    K=1 rank-1 update matmuls; RoPE is applied in the transposed layout via a
    host-side feature permutation that makes every rotation pair a
    partner-lane swap (lane i <-> i+16 within each 32-lane bank) so a single
    DVE stream_shuffle + two muls + one add implements it.
  - v is computed naturally [t, j] with the per-head ones-column folded into
    an extended weight matrix (65 cols/head) so the softmax denominator rides
    the attention-value matmul as row 64 of the output.
  - scores^T = kT_head-stationary @ qT_head (K=64; the two heads of a block
    auto-row-tile at partitions 0/64 and overlap on the PE).
  - exp on ScalarE (scale=1/8 fused, no max subtraction: |scores*scale| < ~3).
  - out^T = [v | 1]-stationary @ P^T accumulated over t; row 64 gives sums;
    reciprocal + shuffle-broadcast + DVE mul normalizes into OT [j, s].
  - final = OT-stationary @ Wo^T + bo, written naturally [s, e] and DMA'd out.
"""
import numpy as np

P = 128
S = 1024
D = 1024
NH = 16
HD = 64
BC = 2            # batches per core
SC = 512          # s-chunk (max fp32 moving free dim)
NT = 8            # 128-tiles per 1024
LATENT = 16
SPATIAL = HD - LATENT
VW = HD + 1       # v columns per head incl. ones column
VD = NH * VW      # 1040
VC = 260          # v projection N-chunk (4 chunks; >=256 keeps fp32r fast)
ROPE_BASE = 10000.0
SCALE = 1.0 / 8.0
SWAP16 = [(i + 16) % 32 for i in range(32)]
N_CORES = 8

_NC_CACHE = None


def _ensure_ntff_hook():
    """Make trace=True work under axon even when antenv.axon_hooks is absent."""
    import sys, types
    try:
        import antenv.axon_hooks  # noqa: F401
    except ImportError:
        try:
            import antenv
        except ImportError:
            return
        mod = types.ModuleType("antenv.axon_hooks")
        mod._hook = None
        def set_axon_ntff_profile_hook(h, _m=mod):
            _m._hook = h
        def get_axon_ntff_profile_hook(_m=mod):
            return _m._hook
        mod.set_axon_ntff_profile_hook = set_axon_ntff_profile_hook
        mod.get_axon_ntff_profile_hook = get_axon_ntff_profile_hook
        sys.modules["antenv.axon_hooks"] = mod
        antenv.axon_hooks = mod
    import antenv.axon_hooks as ah
    if ah.get_axon_ntff_profile_hook() is None:
        try:
            from trn_agent_boot.trn_boot import _ntff_profile_via_ctypes
            hook = _ntff_profile_via_ctypes("/opt/axon/libaxon_pjrt.so")
            if hook is not None:
                ah.set_axon_ntff_profile_hook(hook)
        except Exception:
            pass


def _head_perm():
    """Per-head feature permutation: 32 rotation pairs laid out as two 32-lane
    banks of [16 firsts | 16 seconds] so the partner map is lane i <-> i+16."""
    pairs = []
    for i in range(LATENT // 2):
        pairs.append((2 * i, 2 * i + 1))
    m = SPATIAL // 4
    for j in range(m):
        pairs.append((LATENT + j, LATENT + m + j))
    for j in range(m):
        pairs.append((LATENT + 2 * m + j, LATENT + 3 * m + j))
    perm = []
    for bank in range(2):
        bp = pairs[bank * 16:(bank + 1) * 16]
        perm += [p[0] for p in bp]
        perm += [p[1] for p in bp]
    return np.array(perm)


def _full_perm():
    p = _head_perm()
    return np.concatenate([p + HD * n for n in range(NH)])


def _rope_tables(height, width):
    """Block tables [128, S] (two identical 64-row head replicas) in the
    permuted layout: C[r,s]=cos(angle), Sp[r,s]=±sin(angle)."""
    m1 = LATENT // 2
    inv1 = 1.0 / (ROPE_BASE ** (np.arange(m1) * 2.0 / LATENT))
    ang1 = np.arange(S)[:, None] * inv1[None, :]
    m2 = SPATIAL // 4
    inv2 = 1.0 / (ROPE_BASE ** (np.arange(m2) * 4.0 / SPATIAL))
    xa = np.arange(width)[:, None] * inv2[None, :]
    ya = np.arange(height)[:, None] * inv2[None, :]
    angx = np.broadcast_to(xa[None, :, :], (height, width, m2)).reshape(height * width, m2)
    angy = np.broadcast_to(ya[:, None, :], (height, width, m2)).reshape(height * width, m2)
    ang = np.concatenate([ang1, angx, angy], axis=1)  # (S, 32)
    C = np.zeros((HD, S), np.float32)
    Sp = np.zeros((HD, S), np.float32)
    for r in range(HD):
        b, l = r // 32, r % 32
        pair = b * 16 + (l % 16)
        second = l >= 16
        C[r] = np.cos(ang[:, pair])
        Sp[r] = np.sin(ang[:, pair]) * (1.0 if second else -1.0)
    return (np.concatenate([C, C], axis=0).astype(np.float32),
            np.concatenate([Sp, Sp], axis=0).astype(np.float32))


def _build_nc():
    import concourse.mybir as mybir
    import concourse.tile as tile
    from concourse import bacc

    f32 = mybir.dt.float32
    f32r = mybir.dt.float32r
    AF = mybir.ActivationFunctionType

    nc = bacc.Bacc("TRN2", target_bir_lowering=False, debug=False)
    x_d = nc.dram_tensor("x", [BC * S, D], f32r, kind="ExternalInput").ap()
    wq_d = nc.dram_tensor("wqt", [D, D], f32r, kind="ExternalInput").ap()
    wk_d = nc.dram_tensor("wkt", [D, D], f32r, kind="ExternalInput").ap()
    wv_d = nc.dram_tensor("wvt", [D, VD], f32r, kind="ExternalInput").ap()
    wo_d = nc.dram_tensor("wot", [D, D], f32r, kind="ExternalInput").ap()
    bias_d = nc.dram_tensor("biases", [1, 4 * VD], f32r, kind="ExternalInput").ap()
    ones_d = nc.dram_tensor("onesrow", [1, SC], f32r, kind="ExternalInput").ap()
    ident_d = nc.dram_tensor("ident", [P, P], f32r, kind="ExternalInput").ap()
    tabc_d = nc.dram_tensor("tabc", [P, S], f32, kind="ExternalInput").ap()
    tabs_d = nc.dram_tensor("tabs", [P, S], f32, kind="ExternalInput").ap()
    out_d = nc.dram_tensor("out", [BC * S, D], f32, kind="ExternalOutput").ap()

    with tile.TileContext(nc) as tc:
        from contextlib import ExitStack
        with ExitStack() as ctx:
            pool = lambda name, bufs, space="SBUF": ctx.enter_context(
                tc.tile_pool(name=name, bufs=bufs, space=space))
            consts = pool("consts", 1)
            xnat = pool("xnat", 2)
            xTp = pool("xT", 8)
            qTp = pool("qT", 2)
            kTp = pool("kT", 2)
            vp = pool("v", 8)
            OTp = pool("OT", 8)
            wqp = pool("wq", 9)
            wkp = pool("wk", 9)
            wvp = pool("wv", 9)
            wop = pool("wo", 9)
            pexp = pool("pexp", 3)
            rtmp = pool("rtmp", 3)
            rcp = pool("rcp", 2)
            repp = pool("rep", 2)
            osb = pool("osb", 2)
            psp = pool("ps", 7, space="PSUM")

            ident = consts.tile([P, P], f32r)
            tabc = consts.tile([P, S], f32)
            tabs = consts.tile([P, S], f32)
            biases = consts.tile([1, 4 * VD], f32r)
            ones = consts.tile([1, SC], f32r)
            nc.sync.dma_start(out=ident, in_=ident_d)
            nc.sync.dma_start(out=tabc, in_=tabc_d)
            nc.sync.dma_start(out=tabs, in_=tabs_d)
            nc.sync.dma_start(out=biases, in_=bias_d)
            nc.sync.dma_start(out=ones, in_=ones_d)

            for b in range(BC):
                # ---- Phase A: x^T via PE transpose ----
                xT = [xTp.tile([P, S], f32r, tag="xT", name=f"xT{b}_{i}") for i in range(NT)]
                for st in range(NT):
                    xn = xnat.tile([P, D], f32r, tag="xn")
                    nc.sync.dma_start(
                        out=xn, in_=x_d[b * S + st * P: b * S + (st + 1) * P, :])
                    for dt in range(NT):
                        pst = psp.tile([P, P], f32r, tag="ps")
                        nc.tensor.transpose(pst, xn[:, dt * P:(dt + 1) * P], ident)
                        nc.vector.tensor_copy(
                            xT[dt][:, st * P:(st + 1) * P], pst)

                # ---- Phase B: v (extended with ones cols) ----
                vsb = [vp.tile([P, VD], f32r, tag="v", name=f"vsb{b}_{i}") for i in range(NT)]
                for jc in range(4):
                    wvt = []
                    for dt in range(NT):
                        w = wvp.tile([P, VC], f32r, tag="wv", name=f"wv{b}_{jc}_{dt}")
                        nc.sync.dma_start(
                            out=w, in_=wv_d[dt * P:(dt + 1) * P,
                                            jc * VC:(jc + 1) * VC])
                        wvt.append(w)
                    for tt in range(NT):
                        ps = psp.tile([P, VC], mybir.dt.float32, tag="ps")
                        for dt in range(NT):
                            nc.tensor.matmul(ps, xT[dt][:, tt * P:(tt + 1) * P],
                                             wvt[dt], start=(dt == 0), stop=False)
                        nc.tensor.matmul(
                            ps, ones[:, 0:P],
                            biases[:, 2 * VD + jc * VC: 2 * VD + (jc + 1) * VC],
                            start=False, stop=True)
                        nc.vector.tensor_copy(
                            vsb[tt][:, jc * VC:(jc + 1) * VC], ps)

                # ---- Phase C: per head-pair block ----
                OT = [OTp.tile([P, S], f32r, tag="OT", name=f"OT{b}_{i}") for i in range(NT)]
                for jb in range(NT):
                    qT = qTp.tile([P, S], f32r, tag="qT")
                    kT = kTp.tile([P, S], f32r, tag="kT")
                    wqt, wkt = [], []
                    for dt in range(NT):
                        wq = wqp.tile([P, P], f32r, tag="wq", name=f"wq{b}_{jb}_{dt}")
                        nc.sync.dma_start(
                            out=wq, in_=wq_d[dt * P:(dt + 1) * P,
                                             jb * P:(jb + 1) * P])
                        wqt.append(wq)
                        wk = wkp.tile([P, P], f32r, tag="wk", name=f"wk{b}_{jb}_{dt}")
                        nc.sync.dma_start(
                            out=wk, in_=wk_d[dt * P:(dt + 1) * P,
                                             jb * P:(jb + 1) * P])
                        wkt.append(wk)
                    for sc_i in range(2):
                        scs = slice(sc_i * SC, (sc_i + 1) * SC)
                        for wt, brow, dst in ((wqt, 0, qT), (wkt, 1, kT)):
                            pp = psp.tile([P, SC], mybir.dt.float32, tag="ps")
                            for dt in range(NT):
                                nc.tensor.matmul(pp, wt[dt], xT[dt][:, scs],
                                                 start=(dt == 0), stop=False)
                            nc.tensor.matmul(
                                pp,
                                biases[:, brow * VD + jb * P: brow * VD + (jb + 1) * P],
                                ones, start=False, stop=True)
                            qp = rtmp.tile([P, SC], f32, tag="rt")
                            nc.vector.stream_shuffle(qp, pp, mask=SWAP16)
                            t1 = rtmp.tile([P, SC], f32, tag="rt")
                            nc.vector.tensor_mul(t1, pp, tabc[:, scs])
                            t2 = rtmp.tile([P, SC], f32, tag="rt")
                            nc.vector.tensor_mul(t2, qp, tabs[:, scs])
                            nc.vector.tensor_add(dst[:, scs], t1, t2)

                    # attention for the two heads of this block
                    ps_o = {}
                    for sc_i in range(2):
                        for hh in range(2):
                            ps_o[(hh, sc_i)] = psp.tile(
                                [VW, SC], mybir.dt.float32, tag="ps",
                                name=f"pso{b}_{jb}_{hh}_{sc_i}")
                    for tt in range(NT):
                        for sc_i in range(2):
                            scs = slice(sc_i * SC, (sc_i + 1) * SC)
                            for hh in range(2):
                                r0 = hh * HD
                                n = 2 * jb + hh
                                ps_s = psp.tile([P, SC], mybir.dt.float32,
                                                tag="ps")
                                nc.tensor.matmul(
                                    ps_s, kT[r0:r0 + HD, tt * P:(tt + 1) * P],
                                    qT[r0:r0 + HD, scs], start=True, stop=True)
                                pe = pexp.tile([P, SC], f32r, tag="pe")
                                nc.scalar.activation(pe, ps_s, AF.Exp,
                                                     scale=SCALE)
                                nc.tensor.matmul(
                                    ps_o[(hh, sc_i)],
                                    vsb[tt][:, n * VW:(n + 1) * VW], pe,
                                    start=(tt == 0), stop=(tt == NT - 1))
                    for sc_i in range(2):
                        scs = slice(sc_i * SC, (sc_i + 1) * SC)
                        for hh in range(2):
                            r0 = hh * HD
                            po = ps_o[(hh, sc_i)]
                            rc = rcp.tile([32, SC], f32, tag="rc")
                            nc.gpsimd.memset(rc, 0.0)
                            nc.vector.reciprocal(rc[0:1, :], po[HD:HD + 1, :])
                            rep = repp.tile([HD, SC], f32, tag="rep")
                            nc.vector.stream_shuffle(rep[0:32, :], rc,
                                                     mask=[0] * 32)
                            nc.vector.tensor_copy(rep[32:HD, :], rep[0:32, :])
                            nc.vector.tensor_mul(OT[jb][r0:r0 + HD, scs],
                                                 po[0:HD, :], rep)

                # ---- Phase D: output projection ----
                for ec in range(2):
                    wot = []
                    for jt in range(NT):
                        w = wop.tile([P, SC], f32r, tag="wo", name=f"wo{b}_{ec}_{jt}")
                        nc.sync.dma_start(
                            out=w, in_=wo_d[jt * P:(jt + 1) * P,
                                            ec * SC:(ec + 1) * SC])
                        wot.append(w)
                    for st in range(NT):
                        ps = psp.tile([P, SC], mybir.dt.float32, tag="ps")
                        for jt in range(NT):
                            nc.tensor.matmul(ps, OT[jt][:, st * P:(st + 1) * P],
                                             wot[jt], start=(jt == 0),
                                             stop=False)
                        nc.tensor.matmul(
                            ps, ones[:, 0:P],
                            biases[:, 3 * VD + ec * SC: 3 * VD + (ec + 1) * SC],
                            start=False, stop=True)
                        ob = osb.tile([P, SC], f32, tag="ob")
                        nc.vector.tensor_copy(ob, ps)
                        nc.sync.dma_start(
                            out=out_d[b * S + st * P: b * S + (st + 1) * P,
                                      ec * SC:(ec + 1) * SC], in_=ob)

    nc.compile()
    return nc


def _prep_inputs(x, Wq, bq, Wk, bk, Wv, bv, Wo, bo, height, width):
    """Host-side prep: weight permutation/transposition, RoPE tables, sharding."""
    perm = _full_perm()
    x = np.ascontiguousarray(np.asarray(x, dtype=np.float32))
    wqT = np.ascontiguousarray(np.asarray(Wq, np.float32)[perm].T)
    wkT = np.ascontiguousarray(np.asarray(Wk, np.float32)[perm].T)
    woT = np.ascontiguousarray(np.asarray(Wo, np.float32).T)
    # extended v weights: per head 64 cols + a zero col (ones come from bias)
    WvT = np.asarray(Wv, np.float32).T  # [d, j]
    wvT = np.zeros((D, VD), np.float32)
    bv_ext = np.zeros(VD, np.float32)
    for n in range(NH):
        wvT[:, n * VW: n * VW + HD] = WvT[:, n * HD:(n + 1) * HD]
        bv_ext[n * VW: n * VW + HD] = np.asarray(bv, np.float32)[n * HD:(n + 1) * HD]
        bv_ext[n * VW + HD] = 1.0
    biases = np.zeros((4, VD), np.float32)
    biases[0, :D] = np.asarray(bq, np.float32)[perm]
    biases[1, :D] = np.asarray(bk, np.float32)[perm]
    biases[2] = bv_ext
    biases[3, :D] = np.asarray(bo, np.float32)
    biases = biases.reshape(1, 4 * VD)
    tabc, tabs = _rope_tables(int(height), int(width))
    shared = {
        "wqt": wqT, "wkt": wkT, "wvt": wvT, "wot": woT,
        "biases": biases,
        "onesrow": np.ones((1, SC), np.float32),
        "ident": np.eye(P, dtype=np.float32),
        "tabc": tabc, "tabs": tabs,
    }
    in_maps = []
    for c in range(N_CORES):
        m = dict(shared)
        m["x"] = np.ascontiguousarray(
            x[c * BC:(c + 1) * BC].reshape(BC * S, D))
        in_maps.append(m)
    return in_maps


def _get_nc():
    global _NC_CACHE
    if _NC_CACHE is None:
        _NC_CACHE = _build_nc()
    return _NC_CACHE


def _run(inputs, trace=False):
    _ensure_ntff_hook()
    from concourse.bass_utils import run_bass_kernel_spmd
    in_maps = _prep_inputs(**inputs)
    nc = _get_nc()
    res = run_bass_kernel_spmd(nc, in_maps, core_ids=list(range(N_CORES)),
                               trace=trace)
    B = N_CORES * BC
    out = np.empty((B, S, D), np.float32)
    for c in range(N_CORES):
        out[c * BC:(c + 1) * BC] = res.results[c]["out"].reshape(BC, S, D)
    return out, res


def kernel(**inputs) -> np.ndarray:
    out, _ = _run(inputs, trace=False)
    return out
